# revision 1
# baseline (speedup 1.0000x reference)
"""Trainium2 Bass kernel for the sparse_attention nn.Module problem.

Strategy: data-parallel over the MSA-row dim S (S=128 -> 16 rows per core,
8 cores). All projection weights + pair bias replicated; mask bias and
activations sharded with S. No collectives.

Per-core dataflow (layouts chosen so no on-device input transposes are
needed; host pre-transposes x to [s, c, q] and pre-exponentiates the pair
bias). Matmul chain runs in fp16 (attention weights bf16 for the e^±60
dynamic range); PSUM accumulation is fp32 throughout:
  qT/kT = W @ x^T            (PSUM fp32, DVE evict to fp16; then DMA-remap
                              to a head-flat [d,(h,tc,q)] layout at partition
                              base 0 -- the PE cannot mix tile positions)
  v     = kv_x @ Wv^T        (natural [k, t] layout, evicted to bf16 with a
                              ones column per head for the softmax sum)
  g     = q_x @ Wg^T + bg    (bg added via a rank-1 K=1 matmul; sigmoid done
                              as 0.5*(1+tanh(x/2)) with the 0.5 folded into Wo)
  sT_h  = kT_h^T @ qT_h      (scores transposed: [k, q], per head)
  expS  = exp(sT + mask)     (ACT, mask is the per-partition bias operand;
                              no max-subtraction -- |logits| <= ~70 so
                              fp32->bf16 exp cannot overflow)
  A     = expS * exp(pairT)  (exp(bias_pair) precomputed on host, bf16)
  o,Z   = A_h^T @ [v_h | 1]  (AV matmul in natural layout, N=33 per head;
                              col 32 accumulates Z = sum_k A)
  og    = (tanh(g/2)+1) * (o * (1/Z))
  ogT   = PE transpose(og)
  out   = ogT^T @ (0.5*Wo)^T + bo
"""

import os
import numpy as np
import ml_dtypes

def _mmdt():
    return (ml_dtypes.bfloat16 if os.environ.get('KDTYPE', 'fp16') == 'bf16'
            else np.float16)

B, S, Q, C = 1, 128, 256, 256
H, DH = 8, 32
TOT = H * DH
N_CORES = 8
S_LOC = S // N_CORES  # 16

_CACHE = {}


def _build_program(s_loc):
    import concourse.bacc as bacc
    import concourse.mybir as mybir
    from concourse import tile

    dt = mybir.dt
    f32, bf16 = dt.float32, dt.bfloat16
    f16 = bf16 if os.environ.get('KDTYPE', 'fp16') == 'bf16' else dt.float16
    AF = mybir.ActivationFunctionType
    ALU = mybir.AluOpType

    nc = bacc.Bacc("TRN2", target_bir_lowering=False, debug=False,
                   num_devices=N_CORES)

    x_d = nc.dram_tensor("x", [s_loc, 2 * C, Q], f16, kind="ExternalInput").ap()
    mask_d = nc.dram_tensor("maskt", [128, 2 * s_loc], f32, kind="ExternalInput").ap()
    expb_d = nc.dram_tensor("expb", [128, 2 * H * Q], bf16, kind="ExternalInput").ap()
    wq_d = nc.dram_tensor("wq", [128, 512], f16, kind="ExternalInput").ap()
    wk_d = nc.dram_tensor("wk", [128, 512], f16, kind="ExternalInput").ap()
    wv_d = nc.dram_tensor("wv", [128, 512], f16, kind="ExternalInput").ap()
    wg_d = nc.dram_tensor("wg", [128, 512], f16, kind="ExternalInput").ap()
    wo_d = nc.dram_tensor("wo", [128, 512], f16, kind="ExternalInput").ap()
    bg_d = nc.dram_tensor("bg", [1, 256], f16, kind="ExternalInput").ap()
    bo_d = nc.dram_tensor("bo", [128, 256], f32, kind="ExternalInput").ap()
    id_d = nc.dram_tensor("ident", [128, 128], f16, kind="ExternalInput").ap()
    ones_d = nc.dram_tensor("ones", [1, 128], f16, kind="ExternalInput").ap()
    out_d = nc.dram_tensor("out", [s_loc, Q, C], f32, kind="ExternalOutput").ap()

    def r(ap):
        return ap

    with tile.TileContext(nc) as tc:
        with (
            tc.tile_pool(name="const", bufs=1) as cp,
            tc.tile_pool(name="work", bufs=2) as wp,
            tc.tile_pool(name="ps_small", bufs=2, space="PSUM") as pss,
            tc.tile_pool(name="ps_sc", bufs=2, space="PSUM") as psc,
            tc.tile_pool(name="ps_o", bufs=2, space="PSUM") as pso,
        ):
            # ---- resident constants ----
            wq_t = cp.tile([128, 512], f16, tag="wq")
            wk_t = cp.tile([128, 512], f16, tag="wk")
            wv_t = cp.tile([128, 512], f16, tag="wv")
            wg_t = cp.tile([128, 512], f16, tag="wg")
            wo_t = cp.tile([128, 512], f16, tag="wo")
            expb_t = cp.tile([128, 2 * H * Q], bf16, tag="expb")
            mask_t = cp.tile([128, 2 * s_loc], f32, tag="mask")
            bg_t = cp.tile([1, 256], f16, tag="bg")
            bo_t = cp.tile([128, 256], f32, tag="bo")
            id_t = cp.tile([128, 128], f16, tag="ident")
            ones_t = cp.tile([1, 128], f16, tag="ones")

            nc.sync.dma_start(wq_t[:, :], wq_d[:, :])
            nc.sync.dma_start(wk_t[:, :], wk_d[:, :])
            nc.sync.dma_start(wv_t[:, :], wv_d[:, :])
            nc.sync.dma_start(wg_t[:, :], wg_d[:, :])
            nc.sync.dma_start(wo_t[:, :], wo_d[:, :])
            nc.sync.dma_start(expb_t[:, :], expb_d[:, :])
            nc.sync.dma_start(mask_t[:, :], mask_d[:, :])
            nc.sync.dma_start(bg_t[:, :], bg_d[:, :])
            nc.sync.dma_start(bo_t[:, :], bo_d[:, :])
            nc.sync.dma_start(id_t[:, :], id_d[:, :])
            nc.sync.dma_start(ones_t[:, :], ones_d[:, :])

            for s in range(s_loc):
                # ---- load x^T shards (xq | xkv in one tensor) ----
                xx = wp.tile([128, 1024], f16, tag="xx")
                nc.sync.dma_start(
                    xx[:, :].rearrange("p (cc q) -> p cc q", cc=4),
                    x_d[s].rearrange("(cc p) q -> p cc q", p=128))
                xq = xx[:, 0:512]
                xkv = xx[:, 512:1024]

                # ---- projections (fp32r) ----
                # qT[t, q] += WqT[c, t]^T @ xqT[c, q]
                qt_ps = pss.tile([128, 512], f32, tag="pss")
                for tcc in range(2):
                    for cc in range(2):
                        nc.tensor.matmul(
                            qt_ps[:, tcc * 256:(tcc + 1) * 256],
                            r(wq_t[:, cc * 256 + tcc * 128: cc * 256 + tcc * 128 + 128]),
                            r(xq[:, cc * 256:(cc + 1) * 256]),
                            start=(cc == 0), stop=(cc == 1))
                qt = wp.tile([128, 512], f16, tag="qt")
                ev_q = nc.vector.tensor_copy(qt[:, :], qt_ps[:, :])

                kt_ps = pss.tile([128, 512], f32, tag="pss")
                for tcc in range(2):
                    for cc in range(2):
                        nc.tensor.matmul(
                            kt_ps[:, tcc * 256:(tcc + 1) * 256],
                            r(wk_t[:, cc * 256 + tcc * 128: cc * 256 + tcc * 128 + 128]),
                            r(xkv[:, cc * 256:(cc + 1) * 256]),
                            start=(cc == 0), stop=(cc == 1))
                kt = wp.tile([128, 512], f16, tag="kt")
                ev_k = nc.vector.tensor_copy(kt[:, :], kt_ps[:, :])

                # v natural [k, t]
                v_ps = pss.tile([128, 512], f32, tag="pss")
                for kc in range(2):
                    for cc in range(2):
                        nc.tensor.matmul(
                            v_ps[:, kc * 256:(kc + 1) * 256],
                            r(xkv[:, cc * 256 + kc * 128: cc * 256 + kc * 128 + 128]),
                            r(wv_t[:, cc * 256:(cc + 1) * 256]),
                            start=(cc == 0), stop=(cc == 1))
                # v_aug bf16 [k, (kc, h, 33)]; col 32 of each head = 1.0
                v_sb = wp.tile([128, 528], bf16, tag="v")
                v4 = v_sb.rearrange("p (kc h e) -> p kc h e", kc=2, h=8)
                nc.gpsimd.memset(v4[:, :, :, 32], 1.0)
                for kc in range(2):
                    nc.vector.tensor_copy(
                        v4[:, kc, :, 0:32],
                        v_ps[:, kc * 256:(kc + 1) * 256].rearrange(
                            "p (h d) -> p h d", h=8))

                # g natural [q, t] with bg via rank-1 matmul
                g_ps = pss.tile([128, 512], f32, tag="pss")
                for qc in range(2):
                    for cc in range(2):
                        nc.tensor.matmul(
                            g_ps[:, qc * 256:(qc + 1) * 256],
                            r(xq[:, cc * 256 + qc * 128: cc * 256 + qc * 128 + 128]),
                            r(wg_t[:, cc * 256:(cc + 1) * 256]),
                            start=(cc == 0), stop=False)
                    nc.tensor.matmul(
                        g_ps[:, qc * 256:(qc + 1) * 256],
                        r(ones_t[:, :]), r(bg_t[:, :]),
                        start=False, stop=True)
                # gs = tanh(g/2); sigmoid = 0.5*(gs+1), 0.5 folded into Wo
                gs = wp.tile([128, 512], f32, tag="gs")
                nc.scalar.activation(gs[:, :], g_ps[:, :], AF.Tanh, scale=0.5)

                # ---- attention ----
                # Mixing PE tile positions crashes this runtime, so every
                # matmul must sit at partition base 0: DMA-remap qt/kt from
                # [(hh,d), (tc,q)] to head-flat [d, (tc,hh,q)].
                qt2 = wp.tile([32, 2048], f16, tag="qt2")
                kt2 = wp.tile([32, 2048], f16, tag="kt2")
                # A DMA source AP cannot stride across partitions in a
                # non-leading dim, so remap per head-quarter (both t-chunks
                # in one 3D-AP DMA). Triggers go on otherwise-idle engines
                # to keep the sync queue off the critical path.
                qeng = (nc.gpsimd, nc.scalar, nc.gpsimd, nc.sync)
                keng = (nc.scalar, nc.sync, nc.scalar, nc.sync)
                for hh_ in range(4):
                    srcp = slice(hh_ * 32, hh_ * 32 + 32)
                    dstc = slice(hh_ * 512, hh_ * 512 + 512)
                    qeng[hh_].dma_start(qt2[:, dstc], qt[srcp, :])
                    keng[hh_].dma_start(kt2[:, dstc], kt[srcp, :])

                expS = wp.tile([128, 4096], bf16, tag="expS")
                for hg in range(2):
                    for kc in range(2):
                        sc_ps = psc.tile([128, 1024], f32, tag="sc")
                        for hh in range(4):
                            h = hg * 4 + hh
                            tch, hhh = h // 4, h % 4
                            base = hhh * 512 + tch * 256
                            nc.tensor.matmul(
                                sc_ps[:, hh * 256:(hh + 1) * 256],
                                kt2[:, base + kc * 128: base + kc * 128 + 128],
                                qt2[:, base: base + 256],
                                start=True, stop=True)
                        # exp(s + mask_k) -> bf16
                        nc.scalar.activation(
                            expS[:, kc * 2048 + hg * 1024:
                                 kc * 2048 + (hg + 1) * 1024],
                            sc_ps[:, :], AF.Exp,
                            bias=mask_t[:, kc * s_loc + s: kc * s_loc + s + 1])
                # A = expS * exp(pair bias)
                A = wp.tile([128, 4096], bf16, tag="A")
                for kc in range(2):
                    nc.vector.tensor_mul(
                        A[:, kc * 2048:(kc + 1) * 2048],
                        expS[:, kc * 2048:(kc + 1) * 2048],
                        expb_t[:, kc * 2048:(kc + 1) * 2048])

                # AV: o[q, (h,33)] += A_h^T @ [v_h | 1]
                o_ps = {}
                for qc in range(2):
                    o_ps[qc] = pso.tile([128, 264], f32, tag="o",
                                        name=f"o{qc}")
                for h in range(H):
                    for qc in range(2):
                        for kc in range(2):
                            nc.tensor.matmul(
                                o_ps[qc][:, h * 33: h * 33 + 33],
                                A[:, kc * 2048 + h * 256 + qc * 128:
                                   kc * 2048 + h * 256 + qc * 128 + 128],
                                v_sb[:, kc * 264 + h * 33: kc * 264 + h * 33 + 33],
                                start=(kc == 0), stop=(kc == 1))

                # normalize + gate: og = (gs+1) * (o * (1/Z))
                rz = wp.tile([128, 16], f32, tag="rz")
                t1 = wp.tile([128, 512], f32, tag="t1")
                for qc in range(2):
                    o3 = o_ps[qc].rearrange("p (h e) -> p h e", h=8)
                    nc.vector.reciprocal(
                        rz[:, qc * 8:(qc + 1) * 8], o3[:, :, 32])
                    nc.vector.tensor_mul(
                        t1[:, qc * 256:(qc + 1) * 256].rearrange(
                            "p (h d) -> p h d", h=8),
                        o3[:, :, 0:32],
                        rz[:, qc * 8:(qc + 1) * 8].unsqueeze(2).broadcast_to(
                            (128, 8, 32)))
                og = wp.tile([128, 512], f16, tag="og")
                nc.vector.scalar_tensor_tensor(
                    og[:, :], gs[:, :], 1.0, t1[:, :],
                    op0=ALU.add, op1=ALU.mult)

                # transpose og -> ogT via PE
                tr_ps = pss.tile([128, 512], f16, tag="pss")
                for tcc in range(2):
                    for qc in range(2):
                        nc.tensor.transpose(
                            tr_ps[:, tcc * 256 + qc * 128: tcc * 256 + qc * 128 + 128],
                            og[:, qc * 256 + tcc * 128: qc * 256 + tcc * 128 + 128],
                            id_t[:, :])
                ogt = wp.tile([128, 512], f16, tag="ogt")
                nc.vector.tensor_copy(ogt[:, :], tr_ps[:, :])

                # final projection + bo
                f_ps = pss.tile([128, 512], f32, tag="pss")
                for qc in range(2):
                    for tcc in range(2):
                        nc.tensor.matmul(
                            f_ps[:, qc * 256:(qc + 1) * 256],
                            r(ogt[:, tcc * 256 + qc * 128: tcc * 256 + qc * 128 + 128]),
                            r(wo_t[:, tcc * 256:(tcc + 1) * 256]),
                            start=(tcc == 0), stop=(tcc == 1))
                out_sb = wp.tile([128, 512], f32, tag="out")
                nc.vector.tensor_tensor(
                    out_sb[:, :].rearrange("p (qc c) -> p qc c", qc=2),
                    f_ps[:, :].rearrange("p (qc c) -> p qc c", qc=2),
                    bo_t[:, :].unsqueeze(1).broadcast_to((128, 2, 256)),
                    op=ALU.add)
                nc.sync.dma_start(
                    out_d[s].rearrange("(qc p) c -> p qc c", p=128),
                    out_sb[:, :].rearrange("p (qc c) -> p qc c", qc=2))

    nc.compile()
    return nc


def get_program(s_loc=S_LOC):
    key = (s_loc, os.environ.get('KDTYPE', 'bf16'))
    if key not in _CACHE:
        _CACHE[key] = _build_program(s_loc)
    return _CACHE[key]


def prep_inputs(q_x, kv_x, bias_mask, bias_pair, Wq, Wk, Wv, Wg, bg, Wo, bo,
                s_loc=S_LOC, n_cores=N_CORES):
    """Host-side layout prep. Returns per-core in_maps."""
    bf16 = ml_dtypes.bfloat16

    def wprep(wt):  # (C_in, T_out) -> [p, (cc, t)]
        return np.ascontiguousarray(
            wt.reshape(2, 128, 256).transpose(1, 0, 2).reshape(128, 512)
        ).astype(_mmdt())

    wq_h = wprep(np.asarray(Wq).T)     # lhsT[c, t] = Wq[t, c]
    wk_h = wprep(np.asarray(Wk).T)
    wv_h = wprep(np.asarray(Wv).T)     # rhs[c, t]
    wg_h = wprep(np.asarray(Wg).T)
    wo_h = wprep(np.asarray(Wo).T * 0.5)  # rhs[t, c] = Wo[c, t]; 0.5 = sigmoid fold
    bg_h = np.asarray(bg, _mmdt()).reshape(1, 256)
    bo_h = np.ascontiguousarray(np.broadcast_to(
        np.asarray(bo, np.float32), (128, 256)))
    id_h = np.eye(128, dtype=_mmdt())

    eb = np.exp(np.asarray(bias_pair[0, 0], np.float64)).astype(np.float32)
    ebT = eb.transpose(0, 2, 1)  # (H, K, Q)
    expb_h = np.ascontiguousarray(
        ebT.reshape(H, 2, 128, Q).transpose(2, 1, 0, 3).reshape(128, 2 * H * Q)
    ).astype(bf16)

    x_all = np.concatenate([
        np.asarray(q_x[0], _mmdt()).transpose(0, 2, 1),
        np.asarray(kv_x[0], _mmdt()).transpose(0, 2, 1)], axis=1)
    x_all = np.ascontiguousarray(x_all)   # (S, 2C, Q): xq | xkv
    mask_all = np.asarray(bias_mask[0, :, 0, 0, :], np.float32)  # (S, K)

    in_maps = []
    for core in range(n_cores):
        lo = core * s_loc
        m = mask_all[lo:lo + s_loc]  # (s_loc, K)
        mask_h = np.ascontiguousarray(
            m.T.reshape(2, 128, s_loc).transpose(1, 0, 2).reshape(128, 2 * s_loc))
        in_maps.append({
            "x": x_all[lo:lo + s_loc],
            "maskt": mask_h,
            "expb": expb_h,
            "wq": wq_h, "wk": wk_h, "wv": wv_h, "wg": wg_h, "wo": wo_h,
            "bg": bg_h, "bo": bo_h, "ident": id_h,
            "ones": np.ones((1, 128), _mmdt()),
        })
    return in_maps


def kernel(q_x, kv_x, bias_mask, bias_pair, Wq, Wk, Wv, Wg, bg, Wo, bo):
    from concourse import bass_utils

    nc = get_program()
    in_maps = prep_inputs(q_x, kv_x, bias_mask, bias_pair,
                          Wq, Wk, Wv, Wg, bg, Wo, bo)
    res = bass_utils.run_bass_kernel_spmd(
        nc, in_maps, core_ids=list(range(N_CORES)))
    out = np.concatenate([res.results[i]["out"] for i in range(N_CORES)], axis=0)
    return out.reshape(B, S, Q, C).astype(np.float32)



# revision 4
# speedup vs baseline: 1.0235x; 1.0235x over previous
"""Trainium2 Bass kernel for the sparse_attention nn.Module problem.

Strategy: data-parallel over the MSA-row dim S (S=128 -> 16 rows per core,
8 cores). All projection weights + pair bias replicated; mask bias and
activations sharded with S. No collectives.

v2 design (vs the v1 baseline's 298us):
  - Scores matmuls (K=DH=32) use 4x PE row-tiling (tile_position=(32*hh,0)):
    the 4 heads of a head-group run concurrently in 32-row bands of the PE
    array, with qT/kT consumed in their natural [(hh,d),(tc,q)] projection
    layout -- no DMA remap to partition base 0.
  - AV and the softmax-denominator (Z) matmuls use 4x PE column-tiling
    (tile_position=(0,32*hh)), producing o and Z TRANSPOSED: [(hh,d),(tc,q)].
    This kills the PE transpose of the gated output: og in [t,q] layout is
    directly the lhsT of the final projection.
  - The gate is also computed transposed (gT = Wg @ x^T) so bg rides as the
    ACT per-partition bias and sigmoid(x)=0.5*(tanh(x/2)+1) with 0.5 folded
    into Wo.
  - exp() runs as one FD=2048 ACTIVATE per kc half (mask bias per-partition,
    partitions = keys); pair-bias exp is a host-precomputed resident tile and
    applied with one bf16 2x DVE multiply per kc (split with GPSIMD).
  - 1/Z via reciprocal_approx_fast (custom DVE op, ~51 ULP, ~5x faster than
    the iterative divide).
  - bo applied via a K=1 rank-1 matmul; final out evicted fp16 by ScalarE.
Engine budget per core (16 rows): ACT ~70us (exp-bound), DVE ~80us,
PE ~65us, GPSIMD ~55us -- everything else hides under those.
"""

import os
import numpy as np
import ml_dtypes

def _mmdt():
    return (ml_dtypes.bfloat16 if os.environ.get('KDTYPE', 'fp16') == 'bf16'
            else np.float16)

B, S, Q, C = 1, 128, 256, 256
H, DH = 8, 32
TOT = H * DH
N_CORES = 8
S_LOC = S // N_CORES  # 16

_CACHE = {}


def _build_program(s_loc):
    import concourse.bacc as bacc
    import concourse.mybir as mybir
    from concourse import tile
    from concourse.alu_op_type import AluOpType as ALU

    dt = mybir.dt
    f32, bf16 = dt.float32, dt.bfloat16
    f16 = bf16 if os.environ.get('KDTYPE', 'fp16') == 'bf16' else dt.float16
    AF = mybir.ActivationFunctionType

    nc = bacc.Bacc("TRN2", target_bir_lowering=False, debug=False,
                   num_devices=N_CORES)

    x_d = nc.dram_tensor("x", [s_loc, 2 * C, Q], f16, kind="ExternalInput").ap()
    mask_d = nc.dram_tensor("maskt", [128, 2 * s_loc], f32, kind="ExternalInput").ap()
    expb_d = nc.dram_tensor("expb", [128, 2 * H * Q], bf16, kind="ExternalInput").ap()
    wq_d = nc.dram_tensor("wq", [128, 512], f16, kind="ExternalInput").ap()
    wk_d = nc.dram_tensor("wk", [128, 512], f16, kind="ExternalInput").ap()
    wv_d = nc.dram_tensor("wv", [128, 512], f16, kind="ExternalInput").ap()
    wg_d = nc.dram_tensor("wg", [128, 512], f16, kind="ExternalInput").ap()
    wo_d = nc.dram_tensor("wo", [128, 512], f16, kind="ExternalInput").ap()
    bgt_d = nc.dram_tensor("bgt", [128, 2], f32, kind="ExternalInput").ap()
    bo_d = nc.dram_tensor("bo", [1, 256], f16, kind="ExternalInput").ap()
    ones1_d = nc.dram_tensor("ones1", [1, 128], f16, kind="ExternalInput").ap()
    ones32_d = nc.dram_tensor("ones32", [128, 32], bf16, kind="ExternalInput").ap()
    out_d = nc.dram_tensor("out", [s_loc, Q, C], f16, kind="ExternalOutput").ap()

    with tile.TileContext(nc) as tc:
        with (
            tc.tile_pool(name="const", bufs=1) as cp,
            tc.tile_pool(name="work", bufs=2) as wp,
            tc.tile_pool(name="ps_small", bufs=2, space="PSUM") as pps,
            tc.tile_pool(name="ps_sc", bufs=1, space="PSUM") as psc,
            tc.tile_pool(name="ps_o", bufs=1, space="PSUM") as pso,
            tc.tile_pool(name="ps_z", bufs=1, space="PSUM") as psz,
        ):
            # ---- resident constants ----
            wq_t = cp.tile([128, 512], f16, tag="wq")
            wk_t = cp.tile([128, 512], f16, tag="wk")
            wv_t = cp.tile([128, 512], f16, tag="wv")
            wg_t = cp.tile([128, 512], f16, tag="wg")
            wo_t = cp.tile([128, 512], f16, tag="wo")
            expb_t = cp.tile([128, 2 * H * Q], bf16, tag="expb")
            mask_t = cp.tile([128, 2 * s_loc], f32, tag="mask")
            bgt_t = cp.tile([128, 2], f32, tag="bgt")
            bo_t = cp.tile([1, 256], f16, tag="bo")
            ones1_t = cp.tile([1, 128], f16, tag="ones1")
            ones32_t = cp.tile([128, 32], bf16, tag="ones32")

            nc.sync.dma_start(wq_t[:, :], wq_d[:, :])
            nc.sync.dma_start(wk_t[:, :], wk_d[:, :])
            nc.sync.dma_start(wv_t[:, :], wv_d[:, :])
            nc.sync.dma_start(wg_t[:, :], wg_d[:, :])
            nc.sync.dma_start(wo_t[:, :], wo_d[:, :])
            nc.sync.dma_start(expb_t[:, :], expb_d[:, :])
            nc.sync.dma_start(mask_t[:, :], mask_d[:, :])
            nc.sync.dma_start(bgt_t[:, :], bgt_d[:, :])
            nc.sync.dma_start(bo_t[:, :], bo_d[:, :])
            nc.sync.dma_start(ones1_t[:, :], ones1_d[:, :])
            nc.sync.dma_start(ones32_t[:, :], ones32_d[:, :])

            for s in range(s_loc):
                # ---- load x^T shard (xq | xkv) ----
                xx = wp.tile([128, 1024], f16, tag="xx")
                nc.sync.dma_start(
                    xx[:, :].rearrange("p (cc q) -> p cc q", cc=4),
                    x_d[s].rearrange("(cc p) q -> p cc q", p=128))
                xq = xx[:, 0:512]
                xkv = xx[:, 512:1024]

                # ---- projections; all outputs [128=(hh,d) or k, (blk, 256)] ----
                # qT/kT/gT: out[(hh,d), (tc,q)] = W.T[c,(tc-block)]^T @ x^T[c,q]
                def projT(w_t, tag):
                    ps = pps.tile([128, 512], f32, tag="pp", name=f"{tag}{s}")
                    for tc_ in range(2):
                        for cc in range(2):
                            nc.tensor.matmul(
                                ps[:, tc_ * 256:(tc_ + 1) * 256],
                                w_t[:, cc * 256 + tc_ * 128:
                                    cc * 256 + tc_ * 128 + 128],
                                xq[:, cc * 256:(cc + 1) * 256],
                                start=(cc == 0), stop=(cc == 1))
                    return ps

                qt_ps = projT(wq_t, "q")
                qt = wp.tile([128, 512], f16, tag="qt")
                nc.vector.tensor_copy(qt[:, :], qt_ps[:, :])

                kt_ps = pps.tile([128, 512], f32, tag="pp", name=f"k{s}")
                for tc_ in range(2):
                    for cc in range(2):
                        nc.tensor.matmul(
                            kt_ps[:, tc_ * 256:(tc_ + 1) * 256],
                            wk_t[:, cc * 256 + tc_ * 128:
                                 cc * 256 + tc_ * 128 + 128],
                            xkv[:, cc * 256:(cc + 1) * 256],
                            start=(cc == 0), stop=(cc == 1))
                kt = wp.tile([128, 512], f16, tag="kt")
                nc.vector.tensor_copy(kt[:, :], kt_ps[:, :])

                # v natural: out[k(kc-blk), (h,d)] = xkv^T[c,k]^T @ Wv^T[c,(h,d)]
                v_ps = pps.tile([128, 512], f32, tag="pp", name=f"v{s}")
                for kc in range(2):
                    for cc in range(2):
                        nc.tensor.matmul(
                            v_ps[:, kc * 256:(kc + 1) * 256],
                            xkv[:, cc * 256 + kc * 128: cc * 256 + kc * 128 + 128],
                            wv_t[:, cc * 256:(cc + 1) * 256],
                            start=(cc == 0), stop=(cc == 1))
                v_sb = wp.tile([128, 512], bf16, tag="v")
                nc.vector.tensor_copy(v_sb[:, :], v_ps[:, :])

                # gT like qT; sigmoid = 0.5*(tanh((g+bg)/2)+1), 0.5 in Wo
                gt_ps = pps.tile([128, 512], f32, tag="pp", name=f"g{s}")
                for tc_ in range(2):
                    for cc in range(2):
                        nc.tensor.matmul(
                            gt_ps[:, tc_ * 256:(tc_ + 1) * 256],
                            wg_t[:, cc * 256 + tc_ * 128:
                                 cc * 256 + tc_ * 128 + 128],
                            xq[:, cc * 256:(cc + 1) * 256],
                            start=(cc == 0), stop=(cc == 1))
                gs = wp.tile([128, 512], f32, tag="gs")
                for tc_ in range(2):
                    nc.scalar.activation(
                        gs[:, tc_ * 256:(tc_ + 1) * 256],
                        gt_ps[:, tc_ * 256:(tc_ + 1) * 256],
                        AF.Tanh, bias=bgt_t[:, tc_:tc_ + 1], scale=0.5)

                # ---- attention ----
                # expS free layout: (kc, hh, tc, q); head h = 4*tc + hh
                expS = wp.tile([128, 4096], bf16, tag="expS")
                A = wp.tile([128, 4096], bf16, tag="A")
                o_ps = pso.tile([128, 512], f32, tag="o", name=f"o{s}")
                z_ps = psz.tile([128, 512], f32, tag="z", name=f"z{s}")

                for kc in range(2):
                    # scores: 4x row-tiled over hh bands; bank = hh
                    sc = psc.tile([128, 2048], f32, tag="sc", name=f"sc{s}_{kc}")
                    for tc_ in range(2):
                        for hh in range(4):
                            nc.tensor.matmul(
                                sc[:, hh * 512 + tc_ * 256:
                                   hh * 512 + tc_ * 256 + 256],
                                kt[hh * 32:hh * 32 + 32,
                                   tc_ * 256 + kc * 128: tc_ * 256 + kc * 128 + 128],
                                qt[hh * 32:hh * 32 + 32, tc_ * 256:(tc_ + 1) * 256],
                                start=True, stop=True,
                                tile_position=(hh * 32, 0))
                    # exp(s + mask_kc) over the whole 4-bank tile
                    nc.scalar.activation(
                        expS[:, kc * 2048:(kc + 1) * 2048],
                        sc[:, :], AF.Exp,
                        bias=mask_t[:, kc * s_loc + s: kc * s_loc + s + 1])
                    # A = expS * exp(pair); split DVE (3/4) + GPSIMD (1/4)
                    nc.vector.tensor_mul(
                        A[:, kc * 2048: kc * 2048 + 1536],
                        expS[:, kc * 2048: kc * 2048 + 1536],
                        expb_t[:, kc * 2048: kc * 2048 + 1536])
                    nc.gpsimd.tensor_mul(
                        A[:, kc * 2048 + 1536: kc * 2048 + 2048],
                        expS[:, kc * 2048 + 1536: kc * 2048 + 2048],
                        expb_t[:, kc * 2048 + 1536: kc * 2048 + 2048])

                # AV + Z, 4x column-tiled over hh; out [(hh,d), (tc,q)].
                # kc is the inner loop: a PSUM bank tolerates only one
                # pending accumulation group at a time, so each (tc,hh)
                # region's group must close before the next opens.
                for tc_ in range(2):
                    for hh in range(4):
                        h = 4 * tc_ + hh
                        for kc in range(2):
                            nc.tensor.matmul(
                                o_ps[hh * 32:hh * 32 + 32,
                                     tc_ * 256:(tc_ + 1) * 256],
                                v_sb[:, kc * 256 + h * 32: kc * 256 + h * 32 + 32],
                                A[:, kc * 2048 + hh * 512 + tc_ * 256:
                                   kc * 2048 + hh * 512 + tc_ * 256 + 256],
                                start=(kc == 0), stop=(kc == 1),
                                tile_position=(0, hh * 32))
                for tc_ in range(2):
                    for hh in range(4):
                        for kc in range(2):
                            nc.tensor.matmul(
                                z_ps[hh * 32:hh * 32 + 32,
                                     tc_ * 256:(tc_ + 1) * 256],
                                ones32_t[:, :],
                                A[:, kc * 2048 + hh * 512 + tc_ * 256:
                                   kc * 2048 + hh * 512 + tc_ * 256 + 256],
                                start=(kc == 0), stop=(kc == 1),
                                tile_position=(0, hh * 32))

                # ---- normalize + gate: og = oT * (1/Z) * (gs+1) ----
                rz = wp.tile([128, 512], f32, tag="rz")
                nc.vector.reciprocal_approx_fast(rz[:, :], z_ps[:, :])
                gz = wp.tile([128, 512], f32, tag="gz")
                nc.vector.scalar_tensor_tensor(
                    gz[:, :], gs[:, :], 1.0, rz[:, :],
                    op0=ALU.add, op1=ALU.mult)
                og = wp.tile([128, 512], f16, tag="og")
                nc.vector.tensor_mul(og[:, :], o_ps[:, :], gz[:, :])

                # ---- final projection y[q,(qc,c)] = og^T @ Wo^T + bo ----
                y_ps = pps.tile([128, 512], f32, tag="pp", name=f"y{s}")
                for qc in range(2):
                    for tc_ in range(2):
                        nc.tensor.matmul(
                            y_ps[:, qc * 256:(qc + 1) * 256],
                            og[:, tc_ * 256 + qc * 128: tc_ * 256 + qc * 128 + 128],
                            wo_t[:, tc_ * 256:(tc_ + 1) * 256],
                            start=(tc_ == 0), stop=False)
                    nc.tensor.matmul(
                        y_ps[:, qc * 256:(qc + 1) * 256],
                        ones1_t[:, :], bo_t[:, :],
                        start=False, stop=True)
                y_sb = wp.tile([128, 512], f16, tag="y")
                nc.scalar.copy(y_sb[:, :], y_ps[:, :])
                nc.sync.dma_start(
                    out_d[s].rearrange("(qc p) c -> p qc c", p=128),
                    y_sb[:, :].rearrange("p (qc c) -> p qc c", qc=2))

    nc.compile()
    return nc


def get_program(s_loc=S_LOC):
    key = (s_loc, os.environ.get('KDTYPE', 'fp16'))
    if key not in _CACHE:
        _CACHE[key] = _build_program(s_loc)
    return _CACHE[key]


def prep_inputs(q_x, kv_x, bias_mask, bias_pair, Wq, Wk, Wv, Wg, bg, Wo, bo,
                s_loc=S_LOC, n_cores=N_CORES):
    """Host-side layout prep. Returns per-core in_maps."""
    bf16 = ml_dtypes.bfloat16

    def wprep(wt):  # (C_in, T_out) -> [p, (cc, t)]
        return np.ascontiguousarray(
            wt.reshape(2, 128, 256).transpose(1, 0, 2).reshape(128, 512)
        ).astype(_mmdt())

    wq_h = wprep(np.asarray(Wq).T)     # lhsT[c, t] = Wq[t, c]
    wk_h = wprep(np.asarray(Wk).T)
    wv_h = wprep(np.asarray(Wv).T)     # rhs[c, t]
    wg_h = wprep(np.asarray(Wg).T)
    # rhs[t, c] = Wo[c, t] * 0.5 (sigmoid-tanh fold)
    wo_h = np.ascontiguousarray(
        (np.asarray(Wo).T * 0.5).reshape(2, 128, 256).transpose(1, 0, 2)
        .reshape(128, 512)).astype(_mmdt())
    # bgT[p, tc] = 0.5*bg[tc*128 + p] (ACT bias; tanh((g+bg)/2))
    bgt_h = np.ascontiguousarray(
        0.5 * np.asarray(bg, np.float32).reshape(2, 128).T)
    bo_h = np.asarray(bo, _mmdt()).reshape(1, 256)

    # expb[p, (kc, hh, tc, q)] = exp(pair[h=4*tc+hh, q, k=kc*128+p])
    eb = np.exp(np.asarray(bias_pair[0, 0], np.float64)).astype(np.float32)
    ebT = eb.transpose(0, 2, 1)  # (H, K, Q)
    expb_h = np.ascontiguousarray(
        ebT.reshape(2, 4, 2, 128, Q).transpose(3, 2, 1, 0, 4).reshape(128, 4096)
    ).astype(bf16)

    x_all = np.concatenate([
        np.asarray(q_x[0], _mmdt()).transpose(0, 2, 1),
        np.asarray(kv_x[0], _mmdt()).transpose(0, 2, 1)], axis=1)
    x_all = np.ascontiguousarray(x_all)   # (S, 2C, Q): xq | xkv
    mask_all = np.asarray(bias_mask[0, :, 0, 0, :], np.float32)  # (S, K)

    in_maps = []
    for core in range(n_cores):
        lo = core * s_loc
        m = mask_all[lo:lo + s_loc]  # (s_loc, K)
        mask_h = np.ascontiguousarray(
            m.T.reshape(2, 128, s_loc).transpose(1, 0, 2).reshape(128, 2 * s_loc))
        in_maps.append({
            "x": x_all[lo:lo + s_loc],
            "maskt": mask_h,
            "expb": expb_h,
            "wq": wq_h, "wk": wk_h, "wv": wv_h, "wg": wg_h, "wo": wo_h,
            "bgt": bgt_h, "bo": bo_h,
            "ones1": np.ones((1, 128), _mmdt()),
            "ones32": np.ones((128, 32), bf16),
        })
    return in_maps


def kernel(q_x, kv_x, bias_mask, bias_pair, Wq, Wk, Wv, Wg, bg, Wo, bo):
    from concourse import bass_utils

    nc = get_program()
    in_maps = prep_inputs(q_x, kv_x, bias_mask, bias_pair,
                          Wq, Wk, Wv, Wg, bg, Wo, bo)
    res = bass_utils.run_bass_kernel_spmd(
        nc, in_maps, core_ids=list(range(N_CORES)))
    out = np.concatenate([res.results[i]["out"] for i in range(N_CORES)], axis=0)
    return out.reshape(B, S, Q, C).astype(np.float32)


# revision 5
# speedup vs baseline: 1.7804x; 1.7395x over previous
"""Trainium2 Bass kernel for the sparse_attention nn.Module problem.

Strategy: data-parallel over the MSA-row dim S (S=128 -> 16 rows per core,
8 cores). All projection weights + pair bias replicated; mask bias and
activations sharded with S. No collectives.

v2 design (vs the v1 baseline's 298us):
  - Scores matmuls (K=DH=32) use 4x PE row-tiling (tile_position=(32*hh,0)):
    the 4 heads of a head-group run concurrently in 32-row bands of the PE
    array, with qT/kT consumed in their natural [(hh,d),(tc,q)] projection
    layout -- no DMA remap to partition base 0.
  - AV and the softmax-denominator (Z) matmuls use 4x PE column-tiling
    (tile_position=(0,32*hh)), producing o and Z TRANSPOSED: [(hh,d),(tc,q)].
    This kills the PE transpose of the gated output: og in [t,q] layout is
    directly the lhsT of the final projection.
  - The gate is also computed transposed (gT = Wg @ x^T) so bg rides as the
    ACT per-partition bias and sigmoid(x)=0.5*(tanh(x/2)+1) with 0.5 folded
    into Wo.
  - exp() runs as one FD=2048 ACTIVATE per kc half (mask bias per-partition,
    partitions = keys); pair-bias exp is a host-precomputed resident tile and
    applied with one bf16 2x DVE multiply per kc (split with GPSIMD).
  - 1/Z via reciprocal_approx_fast (custom DVE op, ~51 ULP, ~5x faster than
    the iterative divide).
  - bo applied via a K=1 rank-1 matmul; final out evicted fp16 by ScalarE.
Engine budget per core (16 rows): ACT ~70us (exp-bound), DVE ~80us,
PE ~65us, GPSIMD ~55us -- everything else hides under those.
"""

import os
import numpy as np
import ml_dtypes

def _mmdt():
    return (ml_dtypes.bfloat16 if os.environ.get('KDTYPE', 'fp16') == 'bf16'
            else np.float16)

B, S, Q, C = 1, 128, 256, 256
H, DH = 8, 32
TOT = H * DH
N_CORES = 8
S_LOC = S // N_CORES  # 16

_CACHE = {}


def _build_program(s_loc):
    import concourse.bacc as bacc
    import concourse.mybir as mybir
    from concourse import tile
    from concourse.alu_op_type import AluOpType as ALU

    dt = mybir.dt
    f32, bf16 = dt.float32, dt.bfloat16
    f16 = bf16 if os.environ.get('KDTYPE', 'fp16') == 'bf16' else dt.float16
    AF = mybir.ActivationFunctionType

    nc = bacc.Bacc("TRN2", target_bir_lowering=False, debug=False,
                   num_devices=N_CORES)

    x_d = nc.dram_tensor("x", [s_loc, 2 * C, Q], f16, kind="ExternalInput").ap()
    mask_d = nc.dram_tensor("maskt", [128, 2 * s_loc], f32, kind="ExternalInput").ap()
    expb_d = nc.dram_tensor("expb", [128, 2 * H * Q], bf16, kind="ExternalInput").ap()
    wq_d = nc.dram_tensor("wq", [128, 512], f16, kind="ExternalInput").ap()
    wk_d = nc.dram_tensor("wk", [128, 512], f16, kind="ExternalInput").ap()
    wv_d = nc.dram_tensor("wv", [128, 512], f16, kind="ExternalInput").ap()
    wg_d = nc.dram_tensor("wg", [128, 512], f16, kind="ExternalInput").ap()
    wo_d = nc.dram_tensor("wo", [128, 512], f16, kind="ExternalInput").ap()
    bgt_d = nc.dram_tensor("bgt", [128, 2], f32, kind="ExternalInput").ap()
    bo_d = nc.dram_tensor("bo", [1, 256], f16, kind="ExternalInput").ap()
    ones1_d = nc.dram_tensor("ones1", [1, 128], f16, kind="ExternalInput").ap()
    ones32_d = nc.dram_tensor("ones32", [128, 32], bf16, kind="ExternalInput").ap()
    out_d = nc.dram_tensor("out", [s_loc, Q, C], f16, kind="ExternalOutput").ap()

    with tile.TileContext(nc) as tc:
        with (
            tc.tile_pool(name="const", bufs=1) as cp,
            tc.tile_pool(name="work", bufs=3) as wp,
            tc.tile_pool(name="ps_small", bufs=2, space="PSUM") as pps,
            tc.tile_pool(name="ps_sc", bufs=1, space="PSUM") as psc,
            tc.tile_pool(name="ps_o", bufs=1, space="PSUM") as pso,
            tc.tile_pool(name="ps_z", bufs=1, space="PSUM") as psz,
        ):
            # ---- resident constants ----
            wq_t = cp.tile([128, 512], f16, tag="wq")
            wk_t = cp.tile([128, 512], f16, tag="wk")
            wv_t = cp.tile([128, 512], f16, tag="wv")
            wg_t = cp.tile([128, 512], f16, tag="wg")
            wo_t = cp.tile([128, 512], f16, tag="wo")
            expb_t = cp.tile([128, 2 * H * Q], bf16, tag="expb")
            mask_t = cp.tile([128, 2 * s_loc], f32, tag="mask")
            bgt_t = cp.tile([128, 2], f32, tag="bgt")
            bo_t = cp.tile([1, 256], f16, tag="bo")
            ones1_t = cp.tile([1, 128], f16, tag="ones1")
            ones32_t = cp.tile([128, 32], bf16, tag="ones32")

            nc.sync.dma_start(wq_t[:, :], wq_d[:, :])
            nc.sync.dma_start(wk_t[:, :], wk_d[:, :])
            nc.sync.dma_start(wv_t[:, :], wv_d[:, :])
            nc.sync.dma_start(wg_t[:, :], wg_d[:, :])
            nc.sync.dma_start(wo_t[:, :], wo_d[:, :])
            nc.sync.dma_start(expb_t[:, :], expb_d[:, :])
            nc.sync.dma_start(mask_t[:, :], mask_d[:, :])
            nc.sync.dma_start(bgt_t[:, :], bgt_d[:, :])
            nc.sync.dma_start(bo_t[:, :], bo_d[:, :])
            nc.sync.dma_start(ones1_t[:, :], ones1_d[:, :])
            nc.sync.dma_start(ones32_t[:, :], ones32_d[:, :])

            for s in range(s_loc):
                # ---- load x^T shard (xq | xkv) ----
                xx = wp.tile([128, 1024], f16, tag="xx")
                nc.sync.dma_start(
                    xx[:, :].rearrange("p (cc q) -> p cc q", cc=4),
                    x_d[s].rearrange("(cc p) q -> p cc q", p=128))
                xq = xx[:, 0:512]
                xkv = xx[:, 512:1024]

                # ---- projections; all outputs [128=(hh,d) or k, (blk, 256)] ----
                # qT/kT/gT: out[(hh,d), (tc,q)] = W.T[c,(tc-block)]^T @ x^T[c,q]
                def projT(w_t, tag):
                    ps = pps.tile([128, 512], f32, tag="pp", name=f"{tag}{s}")
                    for tc_ in range(2):
                        for cc in range(2):
                            nc.tensor.matmul(
                                ps[:, tc_ * 256:(tc_ + 1) * 256],
                                w_t[:, cc * 256 + tc_ * 128:
                                    cc * 256 + tc_ * 128 + 128],
                                xq[:, cc * 256:(cc + 1) * 256],
                                start=(cc == 0), stop=(cc == 1))
                    return ps

                qt_ps = projT(wq_t, "q")
                qt = wp.tile([128, 512], f16, tag="qt")
                nc.vector.tensor_copy(qt[:, :], qt_ps[:, :])

                kt_ps = pps.tile([128, 512], f32, tag="pp", name=f"k{s}")
                for tc_ in range(2):
                    for cc in range(2):
                        nc.tensor.matmul(
                            kt_ps[:, tc_ * 256:(tc_ + 1) * 256],
                            wk_t[:, cc * 256 + tc_ * 128:
                                 cc * 256 + tc_ * 128 + 128],
                            xkv[:, cc * 256:(cc + 1) * 256],
                            start=(cc == 0), stop=(cc == 1))
                kt = wp.tile([128, 512], f16, tag="kt")
                nc.vector.tensor_copy(kt[:, :], kt_ps[:, :])

                # v natural: out[k(kc-blk), (h,d)] = xkv^T[c,k]^T @ Wv^T[c,(h,d)]
                v_ps = pps.tile([128, 512], f32, tag="pp", name=f"v{s}")
                for kc in range(2):
                    for cc in range(2):
                        nc.tensor.matmul(
                            v_ps[:, kc * 256:(kc + 1) * 256],
                            xkv[:, cc * 256 + kc * 128: cc * 256 + kc * 128 + 128],
                            wv_t[:, cc * 256:(cc + 1) * 256],
                            start=(cc == 0), stop=(cc == 1))
                v_sb = wp.tile([128, 512], bf16, tag="v")
                nc.vector.tensor_copy(v_sb[:, :], v_ps[:, :])

                # gT like qT; sigmoid = 0.5*(tanh((g+bg)/2)+1), 0.5 in Wo
                gt_ps = pps.tile([128, 512], f32, tag="pp", name=f"g{s}")
                for tc_ in range(2):
                    for cc in range(2):
                        nc.tensor.matmul(
                            gt_ps[:, tc_ * 256:(tc_ + 1) * 256],
                            wg_t[:, cc * 256 + tc_ * 128:
                                 cc * 256 + tc_ * 128 + 128],
                            xq[:, cc * 256:(cc + 1) * 256],
                            start=(cc == 0), stop=(cc == 1))
                gs = wp.tile([128, 512], f32, tag="gs")
                for tc_ in range(2):
                    nc.scalar.activation(
                        gs[:, tc_ * 256:(tc_ + 1) * 256],
                        gt_ps[:, tc_ * 256:(tc_ + 1) * 256],
                        AF.Tanh, bias=bgt_t[:, tc_:tc_ + 1], scale=0.5)

                # ---- attention ----
                # expS free layout: (kc, hh, tc, q); head h = 4*tc + hh
                expS = wp.tile([128, 4096], bf16, tag="expS")
                A = wp.tile([128, 4096], bf16, tag="A")
                o_ps = pso.tile([128, 512], f32, tag="o", name=f"o{s}")
                z_ps = psz.tile([128, 512], f32, tag="z", name=f"z{s}")

                for kc in range(2):
                    # scores: 4x row-tiled over hh bands; bank = hh
                    sc = psc.tile([128, 2048], f32, tag="sc", name=f"sc{s}_{kc}")
                    for tc_ in range(2):
                        for hh in range(4):
                            nc.tensor.matmul(
                                sc[:, hh * 512 + tc_ * 256:
                                   hh * 512 + tc_ * 256 + 256],
                                kt[hh * 32:hh * 32 + 32,
                                   tc_ * 256 + kc * 128: tc_ * 256 + kc * 128 + 128],
                                qt[hh * 32:hh * 32 + 32, tc_ * 256:(tc_ + 1) * 256],
                                start=True, stop=True,
                                tile_position=(hh * 32, 0))
                    # exp(s + mask_kc) over the whole 4-bank tile
                    nc.scalar.activation(
                        expS[:, kc * 2048:(kc + 1) * 2048],
                        sc[:, :], AF.Exp,
                        bias=mask_t[:, kc * s_loc + s: kc * s_loc + s + 1])
                    # A = expS * exp(pair)  (DVE bf16 2x; GPSIMD is kept off
                    # the critical path -- its sem ops cost ~2.2us each)
                    nc.vector.tensor_mul(
                        A[:, kc * 2048:(kc + 1) * 2048],
                        expS[:, kc * 2048:(kc + 1) * 2048],
                        expb_t[:, kc * 2048:(kc + 1) * 2048])

                # AV + Z, 4x column-tiled over hh; out [(hh,d), (tc,q)].
                # kc is the inner loop: a PSUM bank tolerates only one
                # pending accumulation group at a time, so each (tc,hh)
                # region's group must close before the next opens.
                for tc_ in range(2):
                    for hh in range(4):
                        h = 4 * tc_ + hh
                        for kc in range(2):
                            nc.tensor.matmul(
                                o_ps[hh * 32:hh * 32 + 32,
                                     tc_ * 256:(tc_ + 1) * 256],
                                v_sb[:, kc * 256 + h * 32: kc * 256 + h * 32 + 32],
                                A[:, kc * 2048 + hh * 512 + tc_ * 256:
                                   kc * 2048 + hh * 512 + tc_ * 256 + 256],
                                start=(kc == 0), stop=(kc == 1),
                                tile_position=(0, hh * 32))
                for hh in range(4):
                    for kc in range(2):
                        nc.tensor.matmul(
                            z_ps[hh * 32:hh * 32 + 32, 0:512],
                            ones32_t[:, :],
                            A[:, kc * 2048 + hh * 512:
                               kc * 2048 + hh * 512 + 512],
                            start=(kc == 0), stop=(kc == 1),
                            tile_position=(0, hh * 32))

                # ---- normalize + gate: og = oT * (1/Z) * (gs+1) ----
                rz = wp.tile([128, 512], f32, tag="rz")
                nc.vector.reciprocal_approx_fast(rz[:, :], z_ps[:, :])
                gz = wp.tile([128, 512], f32, tag="gz")
                nc.vector.scalar_tensor_tensor(
                    gz[:, :], gs[:, :], 1.0, rz[:, :],
                    op0=ALU.add, op1=ALU.mult)
                og = wp.tile([128, 512], f16, tag="og")
                nc.vector.tensor_mul(og[:, :], o_ps[:, :], gz[:, :])

                # ---- final projection y[q,(qc,c)] = og^T @ Wo^T + bo ----
                y_ps = psz.tile([128, 512], f32, tag="z", name=f"y{s}")
                for qc in range(2):
                    for tc_ in range(2):
                        nc.tensor.matmul(
                            y_ps[:, qc * 256:(qc + 1) * 256],
                            og[:, tc_ * 256 + qc * 128: tc_ * 256 + qc * 128 + 128],
                            wo_t[:, tc_ * 256:(tc_ + 1) * 256],
                            start=(tc_ == 0), stop=False)
                    nc.tensor.matmul(
                        y_ps[:, qc * 256:(qc + 1) * 256],
                        ones1_t[:, :], bo_t[:, :],
                        start=False, stop=True)
                y_sb = wp.tile([128, 512], f16, tag="y")
                nc.scalar.copy(y_sb[:, :], y_ps[:, :])
                nc.sync.dma_start(
                    out_d[s].rearrange("(qc p) c -> p qc c", p=128),
                    y_sb[:, :].rearrange("p (qc c) -> p qc c", qc=2))

    nc.compile()
    return nc


def get_program(s_loc=S_LOC):
    key = (s_loc, os.environ.get('KDTYPE', 'fp16'))
    if key not in _CACHE:
        _CACHE[key] = _build_program(s_loc)
    return _CACHE[key]


def prep_inputs(q_x, kv_x, bias_mask, bias_pair, Wq, Wk, Wv, Wg, bg, Wo, bo,
                s_loc=S_LOC, n_cores=N_CORES):
    """Host-side layout prep. Returns per-core in_maps."""
    bf16 = ml_dtypes.bfloat16

    def wprep(wt):  # (C_in, T_out) -> [p, (cc, t)]
        return np.ascontiguousarray(
            wt.reshape(2, 128, 256).transpose(1, 0, 2).reshape(128, 512)
        ).astype(_mmdt())

    wq_h = wprep(np.asarray(Wq).T)     # lhsT[c, t] = Wq[t, c]
    wk_h = wprep(np.asarray(Wk).T)
    wv_h = wprep(np.asarray(Wv).T)     # rhs[c, t]
    wg_h = wprep(np.asarray(Wg).T)
    # rhs[t, c] = Wo[c, t] * 0.5 (sigmoid-tanh fold)
    wo_h = np.ascontiguousarray(
        (np.asarray(Wo).T * 0.5).reshape(2, 128, 256).transpose(1, 0, 2)
        .reshape(128, 512)).astype(_mmdt())
    # bgT[p, tc] = 0.5*bg[tc*128 + p] (ACT bias; tanh((g+bg)/2))
    bgt_h = np.ascontiguousarray(
        0.5 * np.asarray(bg, np.float32).reshape(2, 128).T)
    bo_h = np.asarray(bo, _mmdt()).reshape(1, 256)

    # expb[p, (kc, hh, tc, q)] = exp(pair[h=4*tc+hh, q, k=kc*128+p])
    eb = np.exp(np.asarray(bias_pair[0, 0], np.float64)).astype(np.float32)
    ebT = eb.transpose(0, 2, 1)  # (H, K, Q)
    expb_h = np.ascontiguousarray(
        ebT.reshape(2, 4, 2, 128, Q).transpose(3, 2, 1, 0, 4).reshape(128, 4096)
    ).astype(bf16)

    x_all = np.concatenate([
        np.asarray(q_x[0], _mmdt()).transpose(0, 2, 1),
        np.asarray(kv_x[0], _mmdt()).transpose(0, 2, 1)], axis=1)
    x_all = np.ascontiguousarray(x_all)   # (S, 2C, Q): xq | xkv
    mask_all = np.asarray(bias_mask[0, :, 0, 0, :], np.float32)  # (S, K)

    in_maps = []
    for core in range(n_cores):
        lo = core * s_loc
        m = mask_all[lo:lo + s_loc]  # (s_loc, K)
        mask_h = np.ascontiguousarray(
            m.T.reshape(2, 128, s_loc).transpose(1, 0, 2).reshape(128, 2 * s_loc))
        in_maps.append({
            "x": x_all[lo:lo + s_loc],
            "maskt": mask_h,
            "expb": expb_h,
            "wq": wq_h, "wk": wk_h, "wv": wv_h, "wg": wg_h, "wo": wo_h,
            "bgt": bgt_h, "bo": bo_h,
            "ones1": np.ones((1, 128), _mmdt()),
            "ones32": np.ones((128, 32), bf16),
        })
    return in_maps


def kernel(q_x, kv_x, bias_mask, bias_pair, Wq, Wk, Wv, Wg, bg, Wo, bo):
    from concourse import bass_utils

    nc = get_program()
    in_maps = prep_inputs(q_x, kv_x, bias_mask, bias_pair,
                          Wq, Wk, Wv, Wg, bg, Wo, bo)
    res = bass_utils.run_bass_kernel_spmd(
        nc, in_maps, core_ids=list(range(N_CORES)))
    out = np.concatenate([res.results[i]["out"] for i in range(N_CORES)], axis=0)
    return out.reshape(B, S, Q, C).astype(np.float32)
